# revision 13
# baseline (speedup 1.0000x reference)
"""Abeles matrix (neutron reflectivity) kernel for 8 Trainium2 NeuronCores.

Algorithm (per point (b,q), L=64 layers):
  k_l = sqrt((q/2)^2 - 4*pi*(sld_l - sld_0)*1e-6 - i*4*pi*1e-9)  (stable branch form)
  r_l = Fresnel(k_l, k_{l+1}) * exp(-2 k_l k_{l+1} sigma_l^2)
  scaled transfer recurrence (u = v / prod(m00), layers descending):
      u0' = u0 + r*u1 ;  u1' = E*(r*u0 + u1),  E = e^{-2 t b}(cos 2ta - i sin 2ta)
  out = |u1/u0|^2

Sharding: pure data-parallel over batch, 32 rows of B=256 per core.
Per-core layout: 128 partitions = 32 b x 4 q-groups, 128 free = q within group.
"""
import sys
sys.path.insert(0, "/opt/trn_rl_repo")
import math
import numpy as np

import concourse.bass as bass
import concourse.mybir as mybir
from concourse import tile
from contextlib import ExitStack

AF = mybir.ActivationFunctionType
ALU = mybir.AluOpType
F32 = mybir.dt.float32
f32 = np.float32

B, Q, L = 256, 512, 64
NCORES = 8
BL = B // NCORES           # 32 batch rows per core
P = 128                    # partitions
QF = 128                   # q elements per partition
CHUNK = 16                 # layers per chunk
DEBUG_DUMP = False

YMAG = 4.0 * math.pi * 1e-9
Y2 = f32(YMAG * YMAG)
LNHALFY = f32(np.log(YMAG / 2.0))
PIO2 = f32(np.pi / 2.0)
INV2PI = f32(1.0 / (2.0 * np.pi))
MAGIC = f32(1.5 * 2.0 ** 23)
CW2PI_1 = f32(6.28125)                  # exact in f32
CW2PI_2 = f32(2.0 * np.pi - 6.28125)

# ---------------------------------------------------------------------------
# Toolchain workarounds for this walrus build:
# 1) InstDrain cannot carry sem waits -> re-emit as sync-engine wait_ge's.
# 2) TensorScalarPtr / Activation-with-AP-scale / CopyPredicated cannot carry
#    sem waits -> strip them onto same-engine wait_ge carrier instructions.
# ---------------------------------------------------------------------------
_PATCHED = False


def _install_patches():
    global _PATCHED
    if _PATCHED:
        return
    _PATCHED = True

    def _handles(tc):
        hm = {}
        for h in tc.sems.allocated().values():
            hm[h.name] = h
        return hm

    def _drain_and_barrier(self, tick_clock, wait_clock):
        nc = self.nc
        drain_inst = nc.sync.drain()
        wait_clock.add_sem_waits(
            drain_inst.ins, tile.ScopedClock({None: tick_clock.global_clock})
        )
        ii = drain_inst.ins
        si = ii.sync_info
        waits = list(si.on_wait) if si is not None else []
        if waits:
            ii.sync_info = mybir.SyncInfo(on_wait=[], on_update=list(si.on_update))
            hm = _handles(self)
            for w in waits:
                h = hm.get(w.ant_name)
                assert h is not None and w.wait_mode == "sem-ge-imm"
                nc.sync.wait_ge(h, w.wait_value)
        nc.all_engine_barrier()
        assert self.sems is not None
        popped = nc._tile_sem_poison_stack.pop()
        assert popped is self._sem_poison
        nc.clear_and_free_semaphores(list(self.sems.allocated().values()))
        nc.all_engine_barrier()

    tile.TileContext._drain_and_barrier = _drain_and_barrier

    _orig_commit = tile.TileContext._commit_instruction

    _STRIP_ALL = (mybir.InstTensorScalarPtr, mybir.InstCopyPredicated,
                  mybir.InstActivation)

    def _commit_instruction(self, inst, lazy_reg_writes=True):
        si = getattr(inst, "sync_info", None)
        if si is not None and si.on_wait:
            waits = list(si.on_wait)
            keep = []
            if not isinstance(inst, _STRIP_ALL):
                # most structs tolerate one wait; strip the rest
                keep = waits[:1]
                waits = waits[1:]
            if waits:
                inst.sync_info = mybir.SyncInfo(on_wait=keep, on_update=list(si.on_update))
                hm = _handles(self)
                eng = self.nc.engines[inst.engine]
                for w in waits:
                    h = hm.get(w.ant_name)
                    assert h is not None and w.wait_mode == "sem-ge-imm", w
                    eng.wait_ge(h, w.wait_value)
        return _orig_commit(self, inst, lazy_reg_writes)

    tile.TileContext._commit_instruction = _commit_instruction


# ---------------------------------------------------------------------------
# Kernel builder (one NeuronCore program; SPMD across 8 cores)
# ---------------------------------------------------------------------------

def _build_kernel():
    _install_patches()
    nc = bass.Bass()

    d_qq = nc.declare_dram_parameter("qq", [P, QF], F32, isOutput=False)
    d_negc = nc.declare_dram_parameter("negc", [P, L + 1], F32, isOutput=False)
    d_dx = nc.declare_dram_parameter("dx", [P, L], F32, isOutput=False)
    d_s2m = nc.declare_dram_parameter("s2m", [P, L], F32, isOutput=False)
    d_s2p = nc.declare_dram_parameter("s2p", [P, L], F32, isOutput=False)
    d_t2 = nc.declare_dram_parameter("t2", [P, L], F32, isOutput=False)
    d_m2t = nc.declare_dram_parameter("m2t", [P, L], F32, isOutput=False)
    d_out = nc.declare_dram_parameter("out", [P, QF], F32, isOutput=True)
    d_dbg = {}
    if DEBUG_DUMP:
        for nm in ("RRE", "RIM", "ERE", "EIMP", "A", "B", "Rlev", "TA2d", "RATREd", "RATIMd", "S2Td", "C2Td", "SWd", "CWd"):
            w = CHUNK * QF if nm not in ("A", "B", "Rlev") else (CHUNK + 1) * QF
            d_dbg[nm] = nc.declare_dram_parameter("dbg_" + nm, [P, w], F32, isOutput=True)

    with tile.TileContext(nc) as tc, ExitStack() as ctx:
        pool = ctx.enter_context(tc.tile_pool(name="sb", bufs=1))

        def tl(name, shape, dtype=F32, bufs=1):
            return pool.tile(shape, dtype, tag=name, name=name, bufs=bufs)

        # persistent inputs
        qq = tl("qq", [P, QF])
        negc = tl("negc", [P, L + 1])
        dxp = tl("dxp", [P, L])
        s2m = tl("s2m", [P, L])
        s2p = tl("s2p", [P, L])
        t2 = tl("t2", [P, L])
        m2t = tl("m2t", [P, L])
        nc.sync.dma_start(qq[:], d_qq[:])
        nc.sync.dma_start(negc[:], d_negc[:])
        nc.sync.dma_start(dxp[:], d_dx[:])
        nc.sync.dma_start(s2m[:], d_s2m[:])
        nc.sync.dma_start(s2p[:], d_s2p[:])
        nc.sync.dma_start(t2[:], d_t2[:])
        nc.sync.dma_start(m2t[:], d_m2t[:])

        # constant bias vectors
        y2b = tl("y2b", [P, 1]);  nc.gpsimd.memset(y2b[:], float(Y2))
        lnhy = tl("lnhy", [P, 1]); nc.gpsimd.memset(lnhy[:], float(LNHALFY))
        pio2 = tl("pio2", [P, 1]); nc.gpsimd.memset(pio2[:], float(PIO2))

        # u state (ping-pong)
        ucur = [tl(f"u{i}_a", [P, QF]) for i in range(4)]   # u0r,u0i,u1r,u1i
        unew = [tl(f"u{i}_b", [P, QF]) for i in range(4)]
        nc.gpsimd.memset(ucur[0][:], 1.0)
        for i in (1, 2, 3):
            nc.gpsimd.memset(ucur[i][:], 0.0)

        CW = CHUNK * QF          # max chunk width (layers)
        CW1 = (CHUNK + 1) * QF   # max chunk width (levels)
        starts = list(range(0, L, CHUNK))
        chunks = [(s0_, min(CHUNK, L - s0_)) for s0_ in starts]

        for l0, CL in reversed(chunks):
            cw = CL * QF
            cw1 = (CL + 1) * QF
            # ---- k levels: X, MSK, R, A, B --------------------------------
            X = tl("X", [P, CW1])
            MSK = tl("MSK", [P, CW1], mybir.dt.uint32)
            R = tl("Rr", [P, CW1])
            SQU = tl("SQU", [P, CW1])   # SQ -> G
            UHB = tl("UHB", [P, CW1])   # AX -> H -> B
            UA = tl("UA", [P, CW1])     # U -> A
            T = tl("T", [P, CW1])
            G = SQU                      # SQ dead once R is computed
            for j in range(CL + 1):
                lv = l0 + j
                sl = slice(j * QF, (j + 1) * QF)
                nc.vector.tensor_scalar(X[:, sl], qq[:], negc[:, lv:lv + 1], None, ALU.add)
                nc.vector.tensor_scalar(MSK[:, sl], X[:, sl], 0.0, None, ALU.is_ge)
            nc.scalar.activation(SQU[:, :cw1], X[:, :cw1], AF.Square)                # SQ = X^2
            nc.scalar.activation(UHB[:, :cw1], X[:, :cw1], AF.Abs)                   # AX = |X|
            nc.scalar.activation(R[:, :cw1], SQU[:, :cw1], AF.Sqrt, bias=y2b[:])     # R = sqrt(X^2+Y2)
            nc.vector.tensor_add(UA[:, :cw1], R[:, :cw1], UHB[:, :cw1])              # U = R + AX
            nc.scalar.activation(T[:, :cw1], UA[:, :cw1], AF.Sqrt, bias=0.0, scale=0.5)   # T = sqrt(U/2)
            nc.scalar.activation(UHB[:, :cw1], UA[:, :cw1], AF.Ln, bias=0.0, scale=0.5)   # H = ln(U/2)
            nc.scalar.activation(G[:, :cw1], UHB[:, :cw1], AF.Exp, bias=lnhy[:], scale=-0.5)  # G=(Y/2)/T
            A = UA; Bt = UHB
            nc.scalar.copy(A[:, :cw1], G[:, :cw1])
            nc.vector.copy_predicated(A[:, :cw1], MSK[:, :cw1], T[:, :cw1])   # A = x>=0 ? T : G
            nc.scalar.copy(Bt[:, :cw1], T[:, :cw1])
            nc.vector.copy_predicated(Bt[:, :cw1], MSK[:, :cw1], G[:, :cw1])  # B = x>=0 ? G : T

            # ---- layer quantities ----------------------------------------
            ac, an = A[:, 0:cw], A[:, QF:cw1]
            bc, bn = Bt[:, 0:cw], Bt[:, QF:cw1]
            rc, rn = R[:, 0:cw], R[:, QF:cw1]
            xc, xn = X[:, 0:cw], X[:, QF:cw1]

            # 13 rotating CW-sized slots (lifetime-disjoint reuse)
            s = [tl(f"s{i}", [P, CW]) for i in range(13)]
            def V(t_):
                return t_[:, :cw]
            P1, P2, AB1, AB2 = s[0], s[1], s[2], s[3]
            nc.vector.tensor_mul(V(P1), ac, an)
            nc.vector.tensor_mul(V(P2), bc, bn)
            nc.vector.tensor_mul(V(AB1), ac, bn)
            nc.vector.tensor_mul(V(AB2), bc, an)
            PR, PP, PI, CI0 = s[4], s[5], s[6], s[7]
            nc.vector.tensor_sub(V(PR), V(P1), V(P2))
            nc.vector.tensor_add(V(PP), V(P1), V(P2))
            nc.vector.tensor_add(V(PI), V(AB1), V(AB2))
            nc.vector.tensor_sub(V(CI0), V(AB1), V(AB2))
            RSUM, SX = s[8], s[9]
            nc.vector.tensor_add(V(RSUM), rc, rn)
            nc.vector.tensor_add(V(SX), xc, xn)
            MAG = s[0]          # P1 dead
            nc.vector.scalar_tensor_tensor(V(MAG), V(PP), 2.0, V(RSUM), ALU.mult, ALU.add)
            LNM, LNR = s[1], s[2]   # P2, AB1 dead
            nc.scalar.activation(V(LNM), V(MAG), AF.Ln)
            nc.scalar.activation(V(LNR), V(RSUM), AF.Ln)
            ARG = s[3]          # AB2 dead
            NUM = s[0]          # MAG dead after LNM
            WI, TA2, TB2 = s[8], s[10], s[11]  # RSUM dead after LNR/MAG
            for j in range(CL):
                lv = l0 + j
                sl = slice(j * QF, (j + 1) * QF)
                # ARG = s2m*PR - ln(mag)
                nc.vector.scalar_tensor_tensor(
                    ARG[:, sl], PR[:, sl], s2m[:, lv:lv + 1], LNM[:, sl], ALU.mult, ALU.subtract)
                nc.vector.tensor_scalar(NUM[:, sl], SX[:, sl], dxp[:, lv:lv + 1], None, ALU.mult)
                nc.vector.tensor_scalar(WI[:, sl], PI[:, sl], s2p[:, lv:lv + 1], None, ALU.mult)
                nc.vector.tensor_scalar(TA2[:, sl], A[:, sl], t2[:, lv:lv + 1], None, ALU.mult)
                nc.vector.tensor_scalar(TB2[:, sl], Bt[:, sl], m2t[:, lv:lv + 1], None, ALU.mult)
            # range-reduce TA2 (up to ~60 rad) into [-pi, pi] for the Sin table
            Ft = s[4]; KK = s[6]     # PR dead after ARG loop, PI dead after WI
            nc.vector.tensor_scalar(V(Ft), V(TA2), float(INV2PI), float(MAGIC), ALU.mult, ALU.add)
            nc.vector.tensor_scalar(V(KK), V(Ft), float(MAGIC), None, ALU.subtract)
            nc.vector.scalar_tensor_tensor(V(Ft), V(KK), float(-CW2PI_1), V(TA2), ALU.mult, ALU.add)
            nc.vector.scalar_tensor_tensor(V(TA2), V(KK), float(-CW2PI_2), V(Ft), ALU.mult, ALU.add)
            ARG2 = s[5]          # PP dead after MAG
            nc.vector.tensor_sub(V(ARG2), V(ARG), V(LNR))
            RR, RR2, EE = s[1], s[2], s[6]   # LNM, LNR dead; PI dead after WI
            nc.scalar.activation(V(RR), V(ARG), AF.Exp)
            nc.scalar.activation(V(RR2), V(ARG2), AF.Exp)
            nc.scalar.activation(V(EE), V(TB2), AF.Exp)
            RATRE, RATIM = s[3], s[4]        # ARG dead after RR, PR dead after ARG loop
            nc.vector.tensor_mul(V(RATRE), V(NUM), V(RR2))
            nc.vector.scalar_tensor_tensor(V(RATIM), V(CI0), 2.0, V(RR), ALU.mult, ALU.mult)
            SW, CWt = s[0], s[5]             # NUM dead after RATRE, ARG2 dead after RR2
            S2T, C2T = s[7], s[12]           # CI0 dead after RATIM, ARG2 dead after RR2
            nc.scalar.activation(V(SW), V(WI), AF.Sin)
            nc.scalar.activation(V(CWt), V(WI), AF.Sin, bias=pio2[:])
            nc.scalar.activation(V(S2T), V(TA2), AF.Sin)
            # cos(TA2): shift by pi/2 then wrap args > pi back by 2*pi
            CSH = s[2]; CMSK = s[11]   # RR2 dead after RATRE, TB2 dead after EE
            nc.vector.tensor_scalar(V(CSH), V(TA2), float(PIO2), None, ALU.add)
            nc.vector.tensor_scalar(V(CMSK), V(CSH), float(np.pi), None, ALU.is_gt)
            nc.vector.scalar_tensor_tensor(V(CSH), V(CMSK), float(-2.0 * np.pi), V(CSH), ALU.mult, ALU.add)
            nc.scalar.activation(V(C2T), V(CSH), AF.Sin)
            # r_nev = RAT * (cw + i sw) ; E = EE*(c2 - i s2)
            RRE = tl("RRE", [P, CW]); RIM = tl("RIM", [P, CW])
            ERE = tl("ERE", [P, CW]); EIMP = tl("EIMP", [P, CW])
            t1_, t2_ = s[8], s[9]            # WI dead after sins, SX dead after NUM
            nc.vector.tensor_mul(V(t1_), V(RATRE), V(CWt))
            nc.vector.tensor_mul(V(t2_), V(RATIM), V(SW))
            nc.vector.tensor_sub(V(RRE), V(t1_), V(t2_))
            nc.vector.tensor_mul(V(t1_), V(RATRE), V(SW))
            nc.vector.tensor_mul(V(t2_), V(RATIM), V(CWt))
            nc.vector.tensor_add(V(RIM), V(t1_), V(t2_))
            nc.vector.tensor_mul(V(ERE), V(EE), V(C2T))
            nc.vector.tensor_mul(V(EIMP), V(EE), V(S2T))

            if DEBUG_DUMP and l0 == 0:
                for nm, tens in (("RRE", RRE), ("RIM", RIM), ("ERE", ERE), ("EIMP", EIMP),
                                 ("A", A), ("B", Bt), ("Rlev", R), ("TA2d", TA2),
                                 ("RATREd", RATRE), ("RATIMd", RATIM),
                                 ("S2Td", S2T), ("C2Td", C2T), ("SWd", SW), ("CWd", CWt)):
                    nc.sync.dma_start(d_dbg[nm][:, :tens.shape[1]], tens[:])
            # ---- sequential update over layers (descending) ---------------
            for j in range(CL - 1, -1, -1):
                sl = slice(j * QF, (j + 1) * QF)
                rre, rim = RRE[:, sl], RIM[:, sl]
                ere, eimp = ERE[:, sl], EIMP[:, sl]
                u0r, u0i, u1r, u1i = (t[:] for t in ucur)
                q1 = tl("q1", [P, QF]); q2 = tl("q2", [P, QF])
                q3 = tl("q3", [P, QF]); q4 = tl("q4", [P, QF])
                sa = tl("sa", [P, QF]); sb_ = tl("sb_", [P, QF])
                nc.vector.tensor_mul(q1[:], rre, u1r)
                nc.vector.tensor_mul(q2[:], rim, u1i)
                nc.vector.tensor_mul(q3[:], rre, u1i)
                nc.vector.tensor_mul(q4[:], rim, u1r)
                nc.vector.tensor_add(sa[:], u0r, q1[:])
                nc.vector.tensor_sub(unew[0][:], sa[:], q2[:])
                nc.vector.tensor_add(sb_[:], u0i, q3[:])
                nc.vector.tensor_add(unew[1][:], sb_[:], q4[:])
                nc.vector.tensor_mul(q1[:], rre, u0r)
                nc.vector.tensor_mul(q2[:], rim, u0i)
                nc.vector.tensor_mul(q3[:], rre, u0i)
                nc.vector.tensor_mul(q4[:], rim, u0r)
                nc.vector.tensor_add(sa[:], u1r, q1[:])
                nc.vector.tensor_sub(sa[:], sa[:], q2[:])       # p1r
                nc.vector.tensor_add(sb_[:], u1i, q3[:])
                nc.vector.tensor_add(sb_[:], sb_[:], q4[:])     # p1i
                nc.vector.tensor_mul(q1[:], ere, sa[:])
                nc.vector.tensor_mul(q2[:], eimp, sb_[:])
                nc.vector.tensor_add(unew[2][:], q1[:], q2[:])
                nc.vector.tensor_mul(q3[:], ere, sb_[:])
                nc.vector.tensor_mul(q4[:], eimp, sa[:])
                nc.vector.tensor_sub(unew[3][:], q3[:], q4[:])
                ucur, unew = unew, ucur

        # ---- epilogue: out = |u1/u0|^2 -----------------------------------
        u0r, u0i, u1r, u1i = (t[:] for t in ucur)
        d1 = tl("q1", [P, QF]); d2 = tl("q2", [P, QF]); d3 = tl("q3", [P, QF])
        nc.scalar.activation(d1[:], u0r, AF.Square)
        nc.scalar.activation(d2[:], u0i, AF.Square)
        nc.vector.tensor_add(d1[:], d1[:], d2[:])      # |u0|^2
        nc.scalar.activation(d2[:], d1[:], AF.Ln)
        nc.scalar.activation(d1[:], d2[:], AF.Exp, bias=0.0, scale=-1.0)  # 1/|u0|^2
        nc.vector.tensor_mul(d2[:], u1r, u0r)
        nc.vector.tensor_mul(d3[:], u1i, u0i)
        nc.vector.tensor_add(d2[:], d2[:], d3[:])
        nc.vector.tensor_mul(d2[:], d2[:], d1[:])      # qr
        OUT = tl("OUT", [P, QF])
        nc.vector.tensor_mul(d3[:], u1i, u0r)
        qi2 = tl("q4", [P, QF])
        nc.vector.tensor_mul(qi2[:], u1r, u0i)
        nc.vector.tensor_sub(d3[:], d3[:], qi2[:])
        nc.vector.tensor_mul(d3[:], d3[:], d1[:])      # qi
        nc.scalar.activation(d2[:], d2[:], AF.Square)
        nc.scalar.activation(d3[:], d3[:], AF.Square)
        nc.vector.tensor_add(OUT[:], d2[:], d3[:])
        nc.sync.dma_start(d_out[:], OUT[:])

    return nc


_NC_CACHE = None


def _get_nc():
    global _NC_CACHE
    if _NC_CACHE is None:
        _NC_CACHE = _build_kernel()
    return _NC_CACHE


def _prep_core_inputs(q, thickness, roughness, sld):
    """Host-side O(B*(Q+L)) prep; returns per-core input dicts."""
    q = q.astype(f32); th = thickness.astype(f32)
    rg = roughness.astype(f32); sld = sld.astype(f32)
    amb = sld[:, 0:1]
    negc64 = -(4.0 * math.pi * 1e-6) * (sld.astype(np.float64) - amb.astype(np.float64))
    negc = negc64.astype(f32)                       # [B, L+1]
    dx = (negc64[:, :-1] - negc64[:, 1:]).astype(f32)  # x_c - x_n, exact in f64
    s2m = (-2.0 * rg * rg).astype(f32)
    s2p = (2.0 * rg * rg).astype(f32)
    t2 = (2.0 * th).astype(f32)
    m2t = (-2.0 * th).astype(f32)
    qq = ((q * f32(0.5)) ** 2).astype(f32)          # [B, Q]

    def rep4(arr):  # [BL, K] -> [128, K] (each row repeated 4x)
        return np.repeat(arr, 4, axis=0).copy()

    in_maps = []
    for c in range(NCORES):
        bs = slice(c * BL, (c + 1) * BL)
        in_maps.append({
            "qq": qq[bs].reshape(P, QF).copy(),
            "negc": rep4(negc[bs]),
            "dx": rep4(dx[bs]),
            "s2m": rep4(s2m[bs]),
            "s2p": rep4(s2p[bs]),
            "t2": rep4(t2[bs]),
            "m2t": rep4(m2t[bs]),
        })
    return in_maps


def run(q, thickness, roughness, sld, trace=False, **trace_kwargs):
    from concourse.bass_utils import run_bass_kernel_spmd
    nc = _get_nc()
    in_maps = _prep_core_inputs(q, thickness, roughness, sld)
    res = run_bass_kernel_spmd(nc, in_maps, core_ids=list(range(NCORES)),
                               trace=trace, **trace_kwargs)
    out = np.empty((B, Q), f32)
    for c in range(NCORES):
        out[c * BL:(c + 1) * BL] = res.results[c]["out"].reshape(BL, Q)
    return out, res


def kernel(q, thickness, roughness, sld):
    out, _ = run(q, thickness, roughness, sld)
    return out
